# revision 40
# baseline (speedup 1.0000x reference)
"""Trainium2 Bass kernel for nn_DeformSpaceAttention (deformable 3x3 unfold +
per-channel max over taps + 1x1 conv + sigmoid).

Strategy (8 cores, data parallel over (batch, H-half)), pixel-partition
layout with PE-matmul bilinear blend:
  - Each core handles one (sample b, 50-row half) shard: 5000 output pixels
    (padded to 5120 = 40 tiles of 128 pixels).
  - Host ships, per core: a zero-padded channels-last fp8e4 y-pair table of
    the full sample (row r = [x(y,x,:) | x(y+1,x,:)] at padded position
    r=(y+8)*116+(x+8); PAD=8 absorbs all out-of-bounds bilinear reads,
    reproducing the reference's zero-padding), int16 gather row indices
    (wrapped-16 layout), and the 4 bilinear corner weights per (pixel, tap).
  - Per (tap, 8-tile group): one SWDGE dma_gather (non-transpose, 1024 idxs
    = the HW limit per call) fetches for each pixel the overlapping fp8
    row-pair [v00|v10|v01|v11] (all 4 bilinear corners x 256 channels,
    1 KB/pixel) into pixel-partition layout [128px, 8, 4, 256].
  - The 4-corner blend runs on the PE: psum[px, c] += diag(m_k) @ g_k with
    bf16 diagonal weight matrices against the fp8 rows.  Diag builds are a
    single tensor_scalar(identity, per-partition scalar) each, distributed
    DVE/Pool/ACT (POOL16/ACT16 of 16 slots) to balance engines.  ACT evicts
    psum -> bf16 in 4-tile batches; DVE keeps a running per-channel max
    over the 9 taps.  Groups are pipelined with one-group lookahead
    (gather and diags for group i+1 are issued before group i's compute).
  - 1x1 conv = DVE multiply by replicated w0 + fold + free-dim reduce,
    interleaved into the last tap per 4-tile chunk; sigmoid(+bias) on ACT;
    store [128, 40] f32; host unshards.

Measured (TimelineSim cost model, the harness metric): 178142 ns vs the
547785 ns session baseline (3.07x); device rel err vs reference 8.0e-3.
"""

import sys
from contextlib import ExitStack

import numpy as np

for _p in ("/opt/pypackages", "/opt/trn_rl_repo"):
    if _p not in sys.path:
        sys.path.append(_p)

import concourse.bass as bass
import concourse.bacc as bacc
import concourse.mybir as mybir
from concourse.bass_utils import run_bass_kernel_spmd
from concourse.masks import make_identity
from concourse.tile import TileContext

F32 = mybir.dt.float32
BF16 = mybir.dt.bfloat16
FP8 = mybir.dt.float8e4
I16 = mybir.dt.int16
ALU = mybir.AluOpType
ACTF = mybir.ActivationFunctionType


class Cfg:
    def __init__(self, H=100, W=100, C=256, PAD=8, n_cores=8, B=4):
        self.H, self.W, self.C, self.PAD = H, W, C, PAD
        self.B = B
        self.n_cores = n_cores
        self.halves = n_cores // B          # shards per sample (2)
        self.RS = H // self.halves          # rows per shard (50)
        self.WP = W + 2 * PAD               # padded row width (116)
        self.HP = H + 2 * PAD
        self.NROWS = self.HP * self.WP      # padded pixel rows (13456)
        self.NPX = self.RS * W              # real pixels per shard (5000)
        self.NBLK = -(-self.NPX // 128)     # pixel tiles of 128 (40)
        self.NPXP = self.NBLK * 128         # padded pixel count (5120)
        self.NM = self.NPXP // 16           # wrapped idx cols (320)
        self.GH = 5                         # gather groups per tap
        self.TPH = self.NBLK // self.GH     # tiles per gather group (8)
        self.NIDX = self.TPH * 128          # idxs per gather call (1024,
        #                                     the SWDGE gather HW limit)
        self.POOL16 = 4                     # of 16 diag slots -> Pool
        self.ACT16 = 2                      # of 16 diag slots -> ACT
        assert C == 256 and self.NBLK % self.GH == 0
        assert self.NROWS < 32767


CFG = Cfg()

KH = (np.arange(9) // 3 - 1).astype(np.float32)
KW = (np.arange(9) % 3 - 1).astype(np.float32)


def build_nc(cfg: Cfg, debug_dump=False):
    """Build the (SPMD, per-core identical) bass program."""
    nc = bacc.Bacc("TRN2", target_bir_lowering=False, debug=False,
                   num_swdge_queues=4, dynamic_dma_scratch_size=49152)
    C = cfg.C
    NROWS = cfg.NROWS
    NBLK, NM = cfg.NBLK, cfg.NM

    xcl = nc.dram_tensor("xcl", [NROWS, 2 * C], FP8, kind="ExternalInput")
    idxd = nc.dram_tensor("idxd", [128, 9, NM], I16, kind="ExternalInput")
    mapd = nc.dram_tensor("mapd", [128, 9, NBLK, 4], F32,
                          kind="ExternalInput")
    w0r = nc.dram_tensor("w0r", [128, C], F32, kind="ExternalInput")
    b0r = nc.dram_tensor("b0r", [128, 1], F32, kind="ExternalInput")
    outd = nc.dram_tensor("out", [128, NBLK], F32, kind="ExternalOutput")
    if debug_dump:
        dbg_acc = nc.dram_tensor("dbg_acc", [128, NBLK, 256], BF16,
                                 kind="ExternalOutput")

    # overlapping row-pair window view for the gather source: row r
    # holds [x(y0,x0)|x(y1,x0)|x(y0,x1)|x(y1,x1)] channel blocks (4C fp8)
    xT_pairs = bass.AP(tensor=xcl.ap().tensor, offset=0,
                       ap=[[2 * C, NROWS - 1], [1, 4 * C]])

    with ExitStack() as ctx, TileContext(nc) as tc:
        with tc.tile_pool(name="const", bufs=1) as pconst:
            identf = pconst.tile([128, 128], F32, name="identf")
            make_identity(nc, identf[:])
            identb = pconst.tile([128, 128], BF16, name="identb")
            nc.vector.tensor_copy(out=identb[:], in_=identf[:])
            w0sb = pconst.tile([128, C], F32, name="w0sb")
            nc.sync.dma_start(out=w0sb[:], in_=w0r.ap())
            w0bf = pconst.tile([128, C], BF16, name="w0bf")
            nc.vector.tensor_copy(out=w0bf[:], in_=w0sb[:])
            b0sb = pconst.tile([128, 1], F32, name="b0sb")
            nc.sync.dma_start(out=b0sb[:], in_=b0r.ap())
            idxs = pconst.tile([128, 9, NM], I16, name="idxs")
            maps = pconst.tile([128, 9, NBLK, 4], F32, name="maps")
            for t in range(9):
                nc.sync.dma_start(out=idxs[:, t], in_=idxd.ap()[:, t])
                nc.sync.dma_start(out=maps[:, t], in_=mapd.ap()[:, t])

            # ------------- gather + PE blend + max -----------------------
            TPH, NIDX = cfg.TPH, cfg.NIDX
            acc = pconst.tile([128, NBLK, 256], BF16, name="acc")
            groups = [(t, h2) for h2 in range(cfg.GH) for t in range(9)]

            with tc.tile_pool(name="pg", bufs=4) as pg, \
                 tc.tile_pool(name="pd", bufs=4) as pd, \
                 tc.tile_pool(name="pev", bufs=3) as pev, \
                 tc.tile_pool(name="pp", bufs=3, space="PSUM") as pp:

                def emit_gather(t, h2, split=1):
                    g = pg.tile([128, TPH, 4, 256], FP8, name="g")
                    m0 = h2 * (NM // cfg.GH)
                    mw = NM // cfg.GH // split
                    for si in range(split):
                        nc.gpsimd.dma_gather(
                            g[:].rearrange("p j k c -> p j (k c)")[
                                :, si * (TPH // split):
                                (si + 1) * (TPH // split)],
                            xT_pairs,
                            idxs[:][:, t, m0 + si * mw:m0 + (si + 1) * mw],
                            NIDX // split, NIDX // split, 4 * C,
                            elem_step=2 * C, transpose=False,
                            queue_num=(t * cfg.GH + h2 + si) % 4)
                    return g

                def emit_diags(t, h2):
                    """Diag builds for one (tap, group)."""
                    dt_ = pd.tile([128, TPH, 4, 128], BF16, name="dt")
                    for jj in range(TPH):
                        j = h2 * TPH + jj
                        for k in range(4):
                            c = (t * NBLK + j) * 4 + k
                            r8 = c % 16
                            if r8 < cfg.POOL16:
                                # min(I, m) == diag(m) for m in [0, 1]:
                                # classified off the slow Multiply path in
                                # the Q7 efficiency table
                                nc.gpsimd.tensor_scalar(
                                    dt_[:, jj, k], identb[:],
                                    maps[:][:, t, j, k:k + 1], None, ALU.min)
                            elif r8 < cfg.POOL16 + cfg.ACT16:
                                nc.scalar.mul(
                                    dt_[:, jj, k], identb[:],
                                    maps[:][:, t, j, k:k + 1])
                            else:
                                nc.vector.tensor_scalar(
                                    dt_[:, jj, k], identb[:],
                                    maps[:][:, t, j, k:k + 1], None, ALU.mult)
                    return dt_

                def emit_compute(t, h2, g, dt_):
                    """Matmuls + evicts + maxes for one (tap, group)."""
                    evg = None
                    if t > 0:
                        evg = pev.tile([128, TPH, 256], BF16, name="evg")
                    for q in range(TPH // 4):
                        pt = pp.tile([128, 4, 256], F32, name="pt",
                                     space="PSUM")
                        for quar in range(4):
                            jj = q * 4 + quar
                            for k in range(4):
                                nc.tensor.matmul(
                                    pt[:, quar], dt_[:, jj, k],
                                    g[:, jj, k],
                                    start=(k == 0), stop=(k == 3))
                        j0 = h2 * TPH + q * 4
                        if t == 0:
                            nc.scalar.activation(
                                out=acc[:][:, j0:j0 + 4], in_=pt[:],
                                func=ACTF.Copy)
                        else:
                            nc.scalar.activation(
                                out=evg[:, q * 4:q * 4 + 4],
                                in_=pt[:], func=ACTF.Copy)
                            nc.vector.tensor_tensor(
                                acc[:][:, j0:j0 + 4], acc[:][:, j0:j0 + 4],
                                evg[:, q * 4:q * 4 + 4], ALU.max)

                cvt = pconst.tile([128, NBLK], F32, name="cvt")

                def emit_conv(h2, q):
                    J0 = h2 * TPH + q * 4
                    sc = pcv2.tile([128, 4, 256], BF16, name="sc")
                    w0b3 = w0bf[:].rearrange(
                        "p (o c) -> p o c", o=1).to_broadcast(
                            [128, 4, 256])
                    nc.vector.tensor_tensor(
                        sc[:], acc[:][:, J0:J0 + 4], w0b3, ALU.mult)
                    fold = pcv2.tile([128, 4, 128], BF16, name="fold")
                    nc.vector.tensor_tensor(
                        fold[:], sc[:][:, :, 0:128], sc[:][:, :, 128:256],
                        ALU.add)
                    nc.vector.tensor_reduce(
                        out=cvt[:, J0:J0 + 4], in_=fold[:],
                        axis=mybir.AxisListType.X, op=ALU.add)

                with tc.tile_pool(name="pcv2", bufs=3) as pcv2:
                    LAG = 1
                    gq = [emit_gather(*groups[0], split=2)]
                    pending = []
                    for gi, (t, h2) in enumerate(groups):
                        if gi + LAG < len(groups):
                            gq.append(emit_gather(*groups[gi + LAG]))
                        pending.append((t, h2, gq.pop(0), emit_diags(t, h2)))
                        if len(pending) > LAG:
                            pr = pending.pop(0)
                            emit_compute(*pr)
                            if pr[0] == 8:
                                for q in range(TPH // 4):
                                    emit_conv(pr[1], q)
                    for pr in pending:
                        emit_compute(*pr)
                        if pr[0] == 8:
                            for q in range(TPH // 4):
                                emit_conv(pr[1], q)

            sg = pconst.tile([128, NBLK], F32, name="sg")
            nc.scalar.activation(out=sg[:], in_=cvt[:], func=ACTF.Sigmoid,
                                 bias=b0sb[:], scale=1.0)
            nc.sync.dma_start(out=outd.ap(), in_=sg[:])

            if debug_dump:
                nc.sync.dma_start(out=dbg_acc.ap(), in_=acc[:])


    nc.compile()
    return nc


def _f32_to_e4m3_u8(a):
    """Round-to-nearest-even f32 -> float8_e4m3fn, returned as uint8 bits."""
    a = np.asarray(a, np.float32)
    try:
        import ml_dtypes
        return a.astype(ml_dtypes.float8_e4m3fn).view(np.uint8)
    except ImportError:
        pass
    # numpy fallback: quantize value, then encode e4m3fn bits
    sign = (a < 0) | ((a == 0) & (np.signbit(a)))
    absa = np.clip(np.abs(a), 0.0, 448.0)
    mant, exp = np.frexp(absa)              # absa = mant * 2**exp
    E = np.maximum(exp - 1, -6)             # value exponent (subnormal floor)
    ulp = np.ldexp(np.float32(1.0), E - 3)
    q = np.round(absa / ulp)                # RNE integer in units of ulp
    val = q * ulp
    m2, e2 = np.frexp(val)
    E2 = e2 - 1
    bits = np.zeros(a.shape, np.uint8)
    normal = (val > 0) & (E2 >= -6)
    sub = (val > 0) & (E2 < -6)
    bits[normal] = (((E2[normal] + 7) << 3)
                    | (np.round(m2[normal] * 16).astype(np.int64) - 8)
                    ).astype(np.uint8)
    bits[sub] = np.round(val[sub] / np.ldexp(np.float32(1.0), -9)
                         ).astype(np.uint8)
    bits[sign] |= 0x80
    return bits


def host_prep(cfg: Cfg, x, offset):
    """Per-core input maps. Core = b * halves + half."""
    H, W, C, PAD, WP = cfg.H, cfg.W, cfg.C, cfg.PAD, cfg.WP
    in_maps = []
    xcl_b = {}
    for b in range(cfg.B):
        pad = np.zeros((cfg.HP, WP, C), np.uint8)
        pad[PAD:PAD + H, PAD:PAD + W] = _f32_to_e4m3_u8(
            np.transpose(x[b], (1, 2, 0)))
        flat = pad.reshape(cfg.NROWS, C)
        pair = np.zeros((cfg.NROWS, 2 * C), np.uint8)
        pair[:, :C] = flat
        pair[:cfg.NROWS - WP, C:] = flat[WP:]
        xcl_b[b] = pair
    for core in range(cfg.n_cores):
        b = core // cfg.halves
        half = core % cfg.halves
        h0 = half * cfg.RS
        npx = cfg.NPXP
        hs = np.full(npx, h0, np.int64)
        ws = np.zeros(npx, np.int64)
        ii = np.arange(cfg.NPX)
        hs[:cfg.NPX] = h0 + ii // W
        ws[:cfg.NPX] = ii % W
        offb = offset[b][:, hs, ws].astype(np.float32)  # [18, npx]
        oy = offb[0::2]                                  # [9, npx]
        ox = offb[1::2]
        iy = np.floor(oy)
        ix = np.floor(ox)
        wy = (oy - iy).astype(np.float32)
        wx = (ox - ix).astype(np.float32)
        ry = hs[None] + PAD + KH[:, None] + iy           # [9, npx]
        cx = np.clip(ws[None] + PAD + KW[:, None] + ix, 0, WP - 2)
        r0 = np.clip(ry, 0, WP - 2)
        idx0 = (r0 * WP + cx).astype(np.int16)           # [9, npx]
        i = np.arange(npx)
        idxd = np.zeros((128, 9, cfg.NM), np.int16)
        for r in range(8):
            idxd[i % 16 + 16 * r, :, i // 16] = idx0.T
        # corner order matches gathered row blocks [v00 | v10 | v01 | v11]
        mapd = np.zeros((128, 9, cfg.NBLK, 4), np.float32)
        mapd[i % 128, :, i // 128, 0] = ((1 - wy) * (1 - wx)).T
        mapd[i % 128, :, i // 128, 1] = (wy * (1 - wx)).T
        mapd[i % 128, :, i // 128, 2] = ((1 - wy) * wx).T
        mapd[i % 128, :, i // 128, 3] = (wy * wx).T
        in_maps.append({
            "xcl": xcl_b[b], "idxd": idxd, "mapd": mapd,
        })
    return in_maps


_NC_CACHE = {}


def get_nc(cfg: Cfg, debug_dump=False):
    key = (cfg.H, cfg.W, cfg.C, cfg.n_cores, debug_dump,
           cfg.POOL16, cfg.ACT16, cfg.GH)
    if key not in _NC_CACHE:
        _NC_CACHE[key] = build_nc(cfg, debug_dump=debug_dump)
    return _NC_CACHE[key]


def kernel(x, offset, w0, b0, trace=False, debug_dump=False):
    cfg = CFG
    x = np.asarray(x, np.float32)
    offset = np.asarray(offset, np.float32)
    w0 = np.asarray(w0, np.float32)
    b0 = np.asarray(b0, np.float32)
    nc = get_nc(cfg, debug_dump=debug_dump)
    in_maps = host_prep(cfg, x, offset)
    w0rep = np.ascontiguousarray(
        np.broadcast_to(w0.reshape(1, cfg.C), (128, cfg.C)), np.float32)
    b0rep = np.full((128, 1), float(b0[0]), np.float32)
    for m in in_maps:
        m["w0r"] = w0rep
        m["b0r"] = b0rep
    if trace:
        try:
            import antenv.axon_hooks  # noqa: F401
        except ImportError:
            trace = False
    res = run_bass_kernel_spmd(nc, in_maps, core_ids=list(range(cfg.n_cores)),
                               trace=trace)
    B, H, W = cfg.B, cfg.H, cfg.W
    out = np.zeros((B, 1, H, W), np.float32)
    for core in range(cfg.n_cores):
        b = core // cfg.halves
        half = core % cfg.halves
        h0 = half * cfg.RS
        o = res.results[core]["out"]              # [128, NBLK]
        o = o.T.reshape(-1)[:cfg.NPX].reshape(cfg.RS, W)
        out[b, 0, h0:h0 + cfg.RS] = o
    if trace or debug_dump:
        kernel.last_results = res
    return out


# revision 42
# speedup vs baseline: 1.0103x; 1.0103x over previous
"""Trainium2 Bass kernel for nn_DeformSpaceAttention (deformable 3x3 unfold +
per-channel max over taps + 1x1 conv + sigmoid).

Strategy (8 cores, data parallel over (batch, H-half)), pixel-partition
layout with PE-matmul bilinear blend:
  - Each core handles one (sample b, 50-row half) shard: 5000 output pixels
    (padded to 5120 = 40 tiles of 128 pixels).
  - Host ships, per core: a zero-padded channels-last fp8e4 y-pair table of
    the full sample (row r = [x(y,x,:) | x(y+1,x,:)] at padded position
    r=(y+8)*116+(x+8); PAD=8 absorbs all out-of-bounds bilinear reads,
    reproducing the reference's zero-padding), int16 gather row indices
    (wrapped-16 layout), and the 4 bilinear corner weights per (pixel, tap).
  - Per (tap, 8-tile group): one SWDGE dma_gather (non-transpose, 1024 idxs
    = the HW limit per call) fetches for each pixel the overlapping fp8
    row-pair [v00|v10|v01|v11] (all 4 bilinear corners x 256 channels,
    1 KB/pixel) into pixel-partition layout [128px, 8, 4, 256].
  - The 4-corner blend runs on the PE: psum[px, c] += diag(m_k) @ g_k with
    bf16 diagonal weight matrices against the fp8 rows.  Diag builds are a
    single tensor_scalar(identity, per-partition scalar) each, distributed
    DVE/Pool/ACT (POOL16/ACT16 of 16 slots) to balance engines.  ACT evicts
    psum -> bf16 in 4-tile batches; DVE keeps a running per-channel max
    over the 9 taps.  Groups are pipelined with one-group lookahead
    (gather and diags for group i+1 are issued before group i's compute).
  - 1x1 conv = DVE multiply by replicated w0 + fold + free-dim reduce,
    interleaved into the last tap per 4-tile chunk; sigmoid(+bias) on ACT;
    store [128, 40] f32; host unshards.

Measured (TimelineSim cost model, the harness metric): 178142 ns vs the
547785 ns session baseline (3.07x); device rel err vs reference 8.0e-3.
"""

import sys
from contextlib import ExitStack

import numpy as np

for _p in ("/opt/pypackages", "/opt/trn_rl_repo"):
    if _p not in sys.path:
        sys.path.append(_p)

import concourse.bass as bass
import concourse.bacc as bacc
import concourse.mybir as mybir
from concourse.bass_utils import run_bass_kernel_spmd
from concourse.masks import make_identity
from concourse.tile import TileContext

F32 = mybir.dt.float32
BF16 = mybir.dt.bfloat16
FP8 = mybir.dt.float8e4
I16 = mybir.dt.int16
ALU = mybir.AluOpType
ACTF = mybir.ActivationFunctionType


class Cfg:
    def __init__(self, H=100, W=100, C=256, PAD=8, n_cores=8, B=4):
        self.H, self.W, self.C, self.PAD = H, W, C, PAD
        self.B = B
        self.n_cores = n_cores
        self.halves = n_cores // B          # shards per sample (2)
        self.RS = H // self.halves          # rows per shard (50)
        self.WP = W + 2 * PAD               # padded row width (116)
        self.HP = H + 2 * PAD
        self.NROWS = self.HP * self.WP      # padded pixel rows (13456)
        self.NPX = self.RS * W              # real pixels per shard (5000)
        self.NBLK = -(-self.NPX // 128)     # pixel tiles of 128 (40)
        self.NPXP = self.NBLK * 128         # padded pixel count (5120)
        self.NM = self.NPXP // 16           # wrapped idx cols (320)
        self.GH = 5                         # gather groups per tap
        self.TPH = self.NBLK // self.GH     # tiles per gather group (8)
        self.NIDX = self.TPH * 128          # idxs per gather call (1024,
        #                                     the SWDGE gather HW limit)
        self.POOL16 = 4                     # of 16 diag slots -> Pool
        self.ACT16 = 2                      # of 16 diag slots -> ACT
        assert C == 256 and self.NBLK % self.GH == 0
        assert self.NROWS < 32767


CFG = Cfg()

KH = (np.arange(9) // 3 - 1).astype(np.float32)
KW = (np.arange(9) % 3 - 1).astype(np.float32)


def build_nc(cfg: Cfg, debug_dump=False):
    """Build the (SPMD, per-core identical) bass program."""
    nc = bacc.Bacc("TRN2", target_bir_lowering=False, debug=False,
                   num_swdge_queues=4, dynamic_dma_scratch_size=49152)
    C = cfg.C
    NROWS = cfg.NROWS
    NBLK, NM = cfg.NBLK, cfg.NM

    xcl = nc.dram_tensor("xcl", [NROWS, 2 * C], FP8, kind="ExternalInput")
    idxd = nc.dram_tensor("idxd", [128, 9, NM], I16, kind="ExternalInput")
    mapd = nc.dram_tensor("mapd", [128, 9, NBLK, 4], F32,
                          kind="ExternalInput")
    w0r = nc.dram_tensor("w0r", [128, C], F32, kind="ExternalInput")
    b0r = nc.dram_tensor("b0r", [128, 1], F32, kind="ExternalInput")
    outd = nc.dram_tensor("out", [128, NBLK], F32, kind="ExternalOutput")
    if debug_dump:
        dbg_acc = nc.dram_tensor("dbg_acc", [128, NBLK, 256], BF16,
                                 kind="ExternalOutput")

    # overlapping row-pair window view for the gather source: row r
    # holds [x(y0,x0)|x(y1,x0)|x(y0,x1)|x(y1,x1)] channel blocks (4C fp8)
    xT_pairs = bass.AP(tensor=xcl.ap().tensor, offset=0,
                       ap=[[2 * C, NROWS - 1], [1, 4 * C]])

    with ExitStack() as ctx, TileContext(nc) as tc:
        with tc.tile_pool(name="const", bufs=1) as pconst:
            identf = pconst.tile([128, 128], F32, name="identf")
            make_identity(nc, identf[:])
            identb = pconst.tile([128, 128], BF16, name="identb")
            nc.vector.tensor_copy(out=identb[:], in_=identf[:])
            w0sb = pconst.tile([128, C], F32, name="w0sb")
            nc.sync.dma_start(out=w0sb[:], in_=w0r.ap())
            w0bf = pconst.tile([128, C], BF16, name="w0bf")
            nc.vector.tensor_copy(out=w0bf[:], in_=w0sb[:])
            b0sb = pconst.tile([128, 1], F32, name="b0sb")
            nc.sync.dma_start(out=b0sb[:], in_=b0r.ap())
            idxs = pconst.tile([128, 9, NM], I16, name="idxs")
            maps = pconst.tile([128, 9, NBLK, 4], F32, name="maps")
            for t in range(9):
                nc.sync.dma_start(out=idxs[:, t], in_=idxd.ap()[:, t])
                nc.sync.dma_start(out=maps[:, t], in_=mapd.ap()[:, t])

            # ------------- gather + PE blend + max -----------------------
            TPH, NIDX = cfg.TPH, cfg.NIDX
            acc = pconst.tile([128, NBLK, 256], BF16, name="acc")
            groups = [(t, h2) for h2 in range(cfg.GH) for t in range(9)]

            with tc.tile_pool(name="pg", bufs=4) as pg, \
                 tc.tile_pool(name="pd", bufs=4) as pd, \
                 tc.tile_pool(name="pev", bufs=3) as pev, \
                 tc.tile_pool(name="pp", bufs=3, space="PSUM") as pp:

                def emit_gather(t, h2, split=1):
                    g = pg.tile([128, TPH, 4, 256], FP8, name="g")
                    m0 = h2 * (NM // cfg.GH)
                    mw = NM // cfg.GH // split
                    for si in range(split):
                        nc.gpsimd.dma_gather(
                            g[:].rearrange("p j k c -> p j (k c)")[
                                :, si * (TPH // split):
                                (si + 1) * (TPH // split)],
                            xT_pairs,
                            idxs[:][:, t, m0 + si * mw:m0 + (si + 1) * mw],
                            NIDX // split, NIDX // split, 4 * C,
                            elem_step=2 * C, transpose=False,
                            queue_num=(t * cfg.GH + h2 + si) % 4)
                    return g

                def emit_diags(t, h2):
                    """Diag builds for one (tap, group)."""
                    dt_ = pd.tile([128, TPH, 4, 128], BF16, name="dt")
                    for jj in range(TPH):
                        j = h2 * TPH + jj
                        for k in range(4):
                            c = (t * NBLK + j) * 4 + k
                            r8 = c % 16
                            if r8 < cfg.POOL16:
                                # min(I, m) == diag(m) for m in [0, 1]:
                                # classified off the slow Multiply path in
                                # the Q7 efficiency table
                                nc.gpsimd.tensor_scalar(
                                    dt_[:, jj, k], identb[:],
                                    maps[:][:, t, j, k:k + 1], None, ALU.min)
                            elif r8 < cfg.POOL16 + cfg.ACT16:
                                nc.scalar.mul(
                                    dt_[:, jj, k], identb[:],
                                    maps[:][:, t, j, k:k + 1])
                            else:
                                nc.vector.tensor_scalar(
                                    dt_[:, jj, k], identb[:],
                                    maps[:][:, t, j, k:k + 1], None, ALU.mult)
                    return dt_

                def emit_compute(t, h2, g, dt_):
                    """Matmuls + evicts + maxes for one (tap, group)."""
                    evg = None
                    if t > 0:
                        evg = pev.tile([128, TPH, 256], BF16, name="evg")
                    for q in range(TPH // 4):
                        pt = pp.tile([128, 4, 256], F32, name="pt",
                                     space="PSUM")
                        for quar in range(4):
                            jj = q * 4 + quar
                            for k in range(4):
                                nc.tensor.matmul(
                                    pt[:, quar], dt_[:, jj, k],
                                    g[:, jj, k],
                                    start=(k == 0), stop=(k == 3))
                        j0 = h2 * TPH + q * 4
                        if t == 0:
                            nc.scalar.activation(
                                out=acc[:][:, j0:j0 + 4], in_=pt[:],
                                func=ACTF.Copy)
                        else:
                            nc.scalar.activation(
                                out=evg[:, q * 4:q * 4 + 4],
                                in_=pt[:], func=ACTF.Copy)
                            nc.vector.tensor_tensor(
                                acc[:][:, j0:j0 + 4], acc[:][:, j0:j0 + 4],
                                evg[:, q * 4:q * 4 + 4], ALU.max)

                cvt = pconst.tile([128, NBLK], F32, name="cvt")

                def emit_conv(h2, q):
                    J0 = h2 * TPH + q * 4
                    sc = pcv2.tile([128, 4, 256], BF16, name="sc")
                    w0b3 = w0bf[:].rearrange(
                        "p (o c) -> p o c", o=1).to_broadcast(
                            [128, 4, 256])
                    nc.vector.tensor_tensor(
                        sc[:], acc[:][:, J0:J0 + 4], w0b3, ALU.mult)
                    fold = pcv2.tile([128, 4, 128], BF16, name="fold")
                    nc.vector.tensor_tensor(
                        fold[:], sc[:][:, :, 0:128], sc[:][:, :, 128:256],
                        ALU.add)
                    nc.vector.tensor_reduce(
                        out=cvt[:, J0:J0 + 4], in_=fold[:],
                        axis=mybir.AxisListType.X, op=ALU.add)

                with tc.tile_pool(name="pcv2", bufs=3) as pcv2:
                    LAG = 1
                    gq = [emit_gather(*groups[0], split=2)]
                    pending = []
                    convq = []

                    def flush_conv(force=False):
                        while convq and (force or len(convq) > 1):
                            h2c = convq.pop(0)
                            for q in range(TPH // 4):
                                emit_conv(h2c, q)

                    for gi, (t, h2) in enumerate(groups):
                        if gi + LAG < len(groups):
                            gq.append(emit_gather(*groups[gi + LAG]))
                        pending.append((t, h2, gq.pop(0), emit_diags(t, h2)))
                        if len(pending) > LAG:
                            pr = pending.pop(0)
                            emit_compute(*pr)
                            flush_conv()
                            if pr[0] == 8:
                                convq.append(pr[1])
                    for pr in pending:
                        emit_compute(*pr)
                        flush_conv()
                        if pr[0] == 8:
                            convq.append(pr[1])
                    flush_conv(force=True)

            sg = pconst.tile([128, NBLK], F32, name="sg")
            nc.scalar.activation(out=sg[:], in_=cvt[:], func=ACTF.Sigmoid,
                                 bias=b0sb[:], scale=1.0)
            nc.sync.dma_start(out=outd.ap(), in_=sg[:])

            if debug_dump:
                nc.sync.dma_start(out=dbg_acc.ap(), in_=acc[:])


    nc.compile()
    return nc


def _f32_to_e4m3_u8(a):
    """Round-to-nearest-even f32 -> float8_e4m3fn, returned as uint8 bits."""
    a = np.asarray(a, np.float32)
    try:
        import ml_dtypes
        return a.astype(ml_dtypes.float8_e4m3fn).view(np.uint8)
    except ImportError:
        pass
    # numpy fallback: quantize value, then encode e4m3fn bits
    sign = (a < 0) | ((a == 0) & (np.signbit(a)))
    absa = np.clip(np.abs(a), 0.0, 448.0)
    mant, exp = np.frexp(absa)              # absa = mant * 2**exp
    E = np.maximum(exp - 1, -6)             # value exponent (subnormal floor)
    ulp = np.ldexp(np.float32(1.0), E - 3)
    q = np.round(absa / ulp)                # RNE integer in units of ulp
    val = q * ulp
    m2, e2 = np.frexp(val)
    E2 = e2 - 1
    bits = np.zeros(a.shape, np.uint8)
    normal = (val > 0) & (E2 >= -6)
    sub = (val > 0) & (E2 < -6)
    bits[normal] = (((E2[normal] + 7) << 3)
                    | (np.round(m2[normal] * 16).astype(np.int64) - 8)
                    ).astype(np.uint8)
    bits[sub] = np.round(val[sub] / np.ldexp(np.float32(1.0), -9)
                         ).astype(np.uint8)
    bits[sign] |= 0x80
    return bits


def host_prep(cfg: Cfg, x, offset):
    """Per-core input maps. Core = b * halves + half."""
    H, W, C, PAD, WP = cfg.H, cfg.W, cfg.C, cfg.PAD, cfg.WP
    in_maps = []
    xcl_b = {}
    for b in range(cfg.B):
        pad = np.zeros((cfg.HP, WP, C), np.uint8)
        pad[PAD:PAD + H, PAD:PAD + W] = _f32_to_e4m3_u8(
            np.transpose(x[b], (1, 2, 0)))
        flat = pad.reshape(cfg.NROWS, C)
        pair = np.zeros((cfg.NROWS, 2 * C), np.uint8)
        pair[:, :C] = flat
        pair[:cfg.NROWS - WP, C:] = flat[WP:]
        xcl_b[b] = pair
    for core in range(cfg.n_cores):
        b = core // cfg.halves
        half = core % cfg.halves
        h0 = half * cfg.RS
        npx = cfg.NPXP
        hs = np.full(npx, h0, np.int64)
        ws = np.zeros(npx, np.int64)
        ii = np.arange(cfg.NPX)
        hs[:cfg.NPX] = h0 + ii // W
        ws[:cfg.NPX] = ii % W
        offb = offset[b][:, hs, ws].astype(np.float32)  # [18, npx]
        oy = offb[0::2]                                  # [9, npx]
        ox = offb[1::2]
        iy = np.floor(oy)
        ix = np.floor(ox)
        wy = (oy - iy).astype(np.float32)
        wx = (ox - ix).astype(np.float32)
        ry = hs[None] + PAD + KH[:, None] + iy           # [9, npx]
        cx = np.clip(ws[None] + PAD + KW[:, None] + ix, 0, WP - 2)
        r0 = np.clip(ry, 0, WP - 2)
        idx0 = (r0 * WP + cx).astype(np.int16)           # [9, npx]
        i = np.arange(npx)
        idxd = np.zeros((128, 9, cfg.NM), np.int16)
        for r in range(8):
            idxd[i % 16 + 16 * r, :, i // 16] = idx0.T
        # corner order matches gathered row blocks [v00 | v10 | v01 | v11]
        mapd = np.zeros((128, 9, cfg.NBLK, 4), np.float32)
        mapd[i % 128, :, i // 128, 0] = ((1 - wy) * (1 - wx)).T
        mapd[i % 128, :, i // 128, 1] = (wy * (1 - wx)).T
        mapd[i % 128, :, i // 128, 2] = ((1 - wy) * wx).T
        mapd[i % 128, :, i // 128, 3] = (wy * wx).T
        in_maps.append({
            "xcl": xcl_b[b], "idxd": idxd, "mapd": mapd,
        })
    return in_maps


_NC_CACHE = {}


def get_nc(cfg: Cfg, debug_dump=False):
    key = (cfg.H, cfg.W, cfg.C, cfg.n_cores, debug_dump,
           cfg.POOL16, cfg.ACT16, cfg.GH)
    if key not in _NC_CACHE:
        _NC_CACHE[key] = build_nc(cfg, debug_dump=debug_dump)
    return _NC_CACHE[key]


def kernel(x, offset, w0, b0, trace=False, debug_dump=False):
    cfg = CFG
    x = np.asarray(x, np.float32)
    offset = np.asarray(offset, np.float32)
    w0 = np.asarray(w0, np.float32)
    b0 = np.asarray(b0, np.float32)
    nc = get_nc(cfg, debug_dump=debug_dump)
    in_maps = host_prep(cfg, x, offset)
    w0rep = np.ascontiguousarray(
        np.broadcast_to(w0.reshape(1, cfg.C), (128, cfg.C)), np.float32)
    b0rep = np.full((128, 1), float(b0[0]), np.float32)
    for m in in_maps:
        m["w0r"] = w0rep
        m["b0r"] = b0rep
    if trace:
        try:
            import antenv.axon_hooks  # noqa: F401
        except ImportError:
            trace = False
    res = run_bass_kernel_spmd(nc, in_maps, core_ids=list(range(cfg.n_cores)),
                               trace=trace)
    B, H, W = cfg.B, cfg.H, cfg.W
    out = np.zeros((B, 1, H, W), np.float32)
    for core in range(cfg.n_cores):
        b = core // cfg.halves
        half = core % cfg.halves
        h0 = half * cfg.RS
        o = res.results[core]["out"]              # [128, NBLK]
        o = o.T.reshape(-1)[:cfg.NPX].reshape(cfg.RS, W)
        out[b, 0, h0:h0 + cfg.RS] = o
    if trace or debug_dump:
        kernel.last_results = res
    return out


# revision 46
# speedup vs baseline: 1.0168x; 1.0063x over previous
"""Trainium2 Bass kernel for nn_DeformSpaceAttention (deformable 3x3 unfold +
per-channel max over taps + 1x1 conv + sigmoid).

Strategy (8 cores, data parallel over (batch, H-half)), pixel-partition
layout with PE-matmul bilinear blend:
  - Each core handles one (sample b, 50-row half) shard: 5000 output pixels
    (padded to 5120 = 40 tiles of 128 pixels).
  - Host ships, per core: a zero-padded channels-last fp8e4 y-pair table of
    the full sample (row r = [x(y,x,:) | x(y+1,x,:)] at padded position
    r=(y+8)*116+(x+8); PAD=8 absorbs all out-of-bounds bilinear reads,
    reproducing the reference's zero-padding), int16 gather row indices
    (wrapped-16 layout), and the 4 bilinear corner weights per (pixel, tap).
  - Per (tap, 8-tile group): one SWDGE dma_gather (non-transpose, 1024 idxs
    = the HW limit per call) fetches for each pixel the overlapping fp8
    row-pair [v00|v10|v01|v11] (all 4 bilinear corners x 256 channels,
    1 KB/pixel) into pixel-partition layout [128px, 8, 4, 256].
  - The 4-corner blend runs on the PE: psum[px, c] += diag(m_k) @ g_k with
    bf16 diagonal weight matrices against the fp8 rows.  Diag builds are a
    single tensor_scalar(identity, per-partition scalar) each, distributed
    DVE/Pool/ACT (POOL16/ACT16 of 16 slots) to balance engines.  ACT evicts
    psum -> bf16 in 4-tile batches; DVE keeps a running per-channel max
    over the 9 taps.  Groups are pipelined with one-group lookahead
    (gather and diags for group i+1 are issued before group i's compute).
  - 1x1 conv = DVE multiply by replicated w0 + fold + free-dim reduce,
    interleaved into the last tap per 4-tile chunk; sigmoid(+bias) on ACT;
    store [128, 40] f32; host unshards.

Measured (TimelineSim cost model, the harness metric): 176318 ns vs the
547785 ns session baseline (3.07x); device rel err vs reference 8.0e-3.
"""

import sys
from contextlib import ExitStack

import numpy as np

for _p in ("/opt/pypackages", "/opt/trn_rl_repo"):
    if _p not in sys.path:
        sys.path.append(_p)

import concourse.bass as bass
import concourse.bacc as bacc
import concourse.mybir as mybir
from concourse.bass_utils import run_bass_kernel_spmd
from concourse.masks import make_identity
from concourse.tile import TileContext

F32 = mybir.dt.float32
BF16 = mybir.dt.bfloat16
FP8 = mybir.dt.float8e4
I16 = mybir.dt.int16
ALU = mybir.AluOpType
ACTF = mybir.ActivationFunctionType


class Cfg:
    def __init__(self, H=100, W=100, C=256, PAD=8, n_cores=8, B=4):
        self.H, self.W, self.C, self.PAD = H, W, C, PAD
        self.B = B
        self.n_cores = n_cores
        self.halves = n_cores // B          # shards per sample (2)
        self.RS = H // self.halves          # rows per shard (50)
        self.WP = W + 2 * PAD               # padded row width (116)
        self.HP = H + 2 * PAD
        self.NROWS = self.HP * self.WP      # padded pixel rows (13456)
        self.NPX = self.RS * W              # real pixels per shard (5000)
        self.NBLK = -(-self.NPX // 128)     # pixel tiles of 128 (40)
        self.NPXP = self.NBLK * 128         # padded pixel count (5120)
        self.NM = self.NPXP // 16           # wrapped idx cols (320)
        self.GH = 5                         # gather groups per tap
        self.TPH = self.NBLK // self.GH     # tiles per gather group (8)
        self.NIDX = self.TPH * 128          # idxs per gather call (1024,
        #                                     the SWDGE gather HW limit)
        self.POOL16 = 4                     # of 16 diag slots -> Pool
        self.ACT16 = 2                      # of 16 diag slots -> ACT
        assert C == 256 and self.NBLK % self.GH == 0
        assert self.NROWS < 32767


CFG = Cfg()

KH = (np.arange(9) // 3 - 1).astype(np.float32)
KW = (np.arange(9) % 3 - 1).astype(np.float32)


def build_nc(cfg: Cfg, debug_dump=False):
    """Build the (SPMD, per-core identical) bass program."""
    nc = bacc.Bacc("TRN2", target_bir_lowering=False, debug=False,
                   num_swdge_queues=4, dynamic_dma_scratch_size=49152)
    C = cfg.C
    NROWS = cfg.NROWS
    NBLK, NM = cfg.NBLK, cfg.NM

    xcl = nc.dram_tensor("xcl", [NROWS, 2 * C], FP8, kind="ExternalInput")
    idxd = nc.dram_tensor("idxd", [128, 9, NM], I16, kind="ExternalInput")
    mapd = nc.dram_tensor("mapd", [128, 9, NBLK, 4], F32,
                          kind="ExternalInput")
    w0r = nc.dram_tensor("w0r", [128, C], F32, kind="ExternalInput")
    b0r = nc.dram_tensor("b0r", [128, 1], F32, kind="ExternalInput")
    outd = nc.dram_tensor("out", [128, NBLK], F32, kind="ExternalOutput")
    if debug_dump:
        dbg_acc = nc.dram_tensor("dbg_acc", [128, NBLK, 256], BF16,
                                 kind="ExternalOutput")

    # overlapping row-pair window view for the gather source: row r
    # holds [x(y0,x0)|x(y1,x0)|x(y0,x1)|x(y1,x1)] channel blocks (4C fp8)
    xT_pairs = bass.AP(tensor=xcl.ap().tensor, offset=0,
                       ap=[[2 * C, NROWS - 1], [1, 4 * C]])

    with ExitStack() as ctx, TileContext(nc) as tc:
        with tc.tile_pool(name="const", bufs=1) as pconst:
            identf = pconst.tile([128, 128], F32, name="identf")
            make_identity(nc, identf[:])
            identb = pconst.tile([128, 128], BF16, name="identb")
            nc.vector.tensor_copy(out=identb[:], in_=identf[:])
            idxs = pconst.tile([128, 9, NM], I16, name="idxs")
            maps = pconst.tile([128, 9, NBLK, 4], F32, name="maps")
            nc.sync.dma_start(out=idxs[:, 0], in_=idxd.ap()[:, 0])
            nc.sync.dma_start(out=maps[:, 0], in_=mapd.ap()[:, 0])
            w0sb = pconst.tile([128, C], F32, name="w0sb")
            nc.sync.dma_start(out=w0sb[:], in_=w0r.ap())
            w0bf = pconst.tile([128, C], BF16, name="w0bf")
            nc.vector.tensor_copy(out=w0bf[:], in_=w0sb[:])
            b0sb = pconst.tile([128, 1], F32, name="b0sb")
            nc.sync.dma_start(out=b0sb[:], in_=b0r.ap())
            for t in range(1, 9):
                nc.sync.dma_start(out=idxs[:, t], in_=idxd.ap()[:, t])
                nc.sync.dma_start(out=maps[:, t], in_=mapd.ap()[:, t])

            # ------------- gather + PE blend + max -----------------------
            TPH, NIDX = cfg.TPH, cfg.NIDX
            acc = pconst.tile([128, NBLK, 256], BF16, name="acc")
            groups = [(t, h2) for h2 in range(cfg.GH) for t in range(9)]

            with tc.tile_pool(name="pg", bufs=4) as pg, \
                 tc.tile_pool(name="pd", bufs=4) as pd, \
                 tc.tile_pool(name="pev", bufs=3) as pev, \
                 tc.tile_pool(name="pp", bufs=3, space="PSUM") as pp:

                def emit_gather(t, h2, split=1):
                    g = pg.tile([128, TPH, 4, 256], FP8, name="g")
                    m0 = h2 * (NM // cfg.GH)
                    mw = NM // cfg.GH // split
                    for si in range(split):
                        nc.gpsimd.dma_gather(
                            g[:].rearrange("p j k c -> p j (k c)")[
                                :, si * (TPH // split):
                                (si + 1) * (TPH // split)],
                            xT_pairs,
                            idxs[:][:, t, m0 + si * mw:m0 + (si + 1) * mw],
                            NIDX // split, NIDX // split, 4 * C,
                            elem_step=2 * C, transpose=False,
                            queue_num=(t * cfg.GH + h2 + si) % 4)
                    return g

                def emit_diags(t, h2):
                    """Diag builds for one (tap, group)."""
                    dt_ = pd.tile([128, TPH, 4, 128], BF16, name="dt")
                    for jj in range(TPH):
                        j = h2 * TPH + jj
                        for k in range(4):
                            c = (t * NBLK + j) * 4 + k
                            r8 = c % 16
                            if r8 < cfg.POOL16:
                                # min(I, m) == diag(m) for m in [0, 1]:
                                # classified off the slow Multiply path in
                                # the Q7 efficiency table
                                nc.gpsimd.tensor_scalar(
                                    dt_[:, jj, k], identb[:],
                                    maps[:][:, t, j, k:k + 1], None, ALU.min)
                            elif r8 < cfg.POOL16 + cfg.ACT16:
                                nc.scalar.mul(
                                    dt_[:, jj, k], identb[:],
                                    maps[:][:, t, j, k:k + 1])
                            else:
                                nc.vector.tensor_scalar(
                                    dt_[:, jj, k], identb[:],
                                    maps[:][:, t, j, k:k + 1], None, ALU.mult)
                    return dt_

                def emit_compute(t, h2, g, dt_):
                    """Matmuls + evicts + maxes for one (tap, group)."""
                    evg = None
                    if t > 0:
                        evg = pev.tile([128, TPH, 256], BF16, name="evg")
                    for q in range(TPH // 4):
                        pt = pp.tile([128, 4, 256], F32, name="pt",
                                     space="PSUM")
                        for quar in range(4):
                            jj = q * 4 + quar
                            for k in range(4):
                                nc.tensor.matmul(
                                    pt[:, quar], dt_[:, jj, k],
                                    g[:, jj, k],
                                    start=(k == 0), stop=(k == 3))
                        j0 = h2 * TPH + q * 4
                        if t == 0:
                            nc.scalar.activation(
                                out=acc[:][:, j0:j0 + 4], in_=pt[:],
                                func=ACTF.Copy)
                        else:
                            nc.scalar.activation(
                                out=evg[:, q * 4:q * 4 + 4],
                                in_=pt[:], func=ACTF.Copy)
                            nc.vector.tensor_tensor(
                                acc[:][:, j0:j0 + 4], acc[:][:, j0:j0 + 4],
                                evg[:, q * 4:q * 4 + 4], ALU.max)

                cvt = pconst.tile([128, NBLK], F32, name="cvt")

                def emit_conv(h2, q):
                    J0 = h2 * TPH + q * 4
                    sc = pcv2.tile([128, 4, 256], BF16, name="sc")
                    w0b3 = w0bf[:].rearrange(
                        "p (o c) -> p o c", o=1).to_broadcast(
                            [128, 4, 256])
                    nc.vector.tensor_tensor(
                        sc[:], acc[:][:, J0:J0 + 4], w0b3, ALU.mult)
                    fold = pcv2.tile([128, 4, 128], BF16, name="fold")
                    nc.vector.tensor_tensor(
                        fold[:], sc[:][:, :, 0:128], sc[:][:, :, 128:256],
                        ALU.add)
                    nc.vector.tensor_reduce(
                        out=cvt[:, J0:J0 + 4], in_=fold[:],
                        axis=mybir.AxisListType.X, op=ALU.add)

                with tc.tile_pool(name="pcv2", bufs=3) as pcv2:
                    LAG = 1
                    gq = [emit_gather(*groups[0], split=2)]
                    pending = []
                    convq = []

                    def flush_conv(force=False):
                        while convq and (force or len(convq) > 1):
                            h2c = convq.pop(0)
                            for q in range(TPH // 4):
                                emit_conv(h2c, q)

                    for gi, (t, h2) in enumerate(groups):
                        if gi + LAG < len(groups):
                            gq.append(emit_gather(*groups[gi + LAG]))
                        pending.append((t, h2, gq.pop(0), emit_diags(t, h2)))
                        if len(pending) > LAG:
                            pr = pending.pop(0)
                            emit_compute(*pr)
                            flush_conv()
                            if pr[0] == 8:
                                convq.append(pr[1])
                    for pr in pending:
                        emit_compute(*pr)
                        flush_conv()
                        if pr[0] == 8:
                            convq.append(pr[1])
                    flush_conv(force=True)

            sg = pconst.tile([128, NBLK], F32, name="sg")
            nc.scalar.activation(out=sg[:], in_=cvt[:], func=ACTF.Sigmoid,
                                 bias=b0sb[:], scale=1.0)
            nc.sync.dma_start(out=outd.ap(), in_=sg[:])

            if debug_dump:
                nc.sync.dma_start(out=dbg_acc.ap(), in_=acc[:])


    nc.compile()
    return nc


def _f32_to_e4m3_u8(a):
    """Round-to-nearest-even f32 -> float8_e4m3fn, returned as uint8 bits."""
    a = np.asarray(a, np.float32)
    try:
        import ml_dtypes
        return a.astype(ml_dtypes.float8_e4m3fn).view(np.uint8)
    except ImportError:
        pass
    # numpy fallback: quantize value, then encode e4m3fn bits
    sign = (a < 0) | ((a == 0) & (np.signbit(a)))
    absa = np.clip(np.abs(a), 0.0, 448.0)
    mant, exp = np.frexp(absa)              # absa = mant * 2**exp
    E = np.maximum(exp - 1, -6)             # value exponent (subnormal floor)
    ulp = np.ldexp(np.float32(1.0), E - 3)
    q = np.round(absa / ulp)                # RNE integer in units of ulp
    val = q * ulp
    m2, e2 = np.frexp(val)
    E2 = e2 - 1
    bits = np.zeros(a.shape, np.uint8)
    normal = (val > 0) & (E2 >= -6)
    sub = (val > 0) & (E2 < -6)
    bits[normal] = (((E2[normal] + 7) << 3)
                    | (np.round(m2[normal] * 16).astype(np.int64) - 8)
                    ).astype(np.uint8)
    bits[sub] = np.round(val[sub] / np.ldexp(np.float32(1.0), -9)
                         ).astype(np.uint8)
    bits[sign] |= 0x80
    return bits


def host_prep(cfg: Cfg, x, offset):
    """Per-core input maps. Core = b * halves + half."""
    H, W, C, PAD, WP = cfg.H, cfg.W, cfg.C, cfg.PAD, cfg.WP
    in_maps = []
    xcl_b = {}
    for b in range(cfg.B):
        pad = np.zeros((cfg.HP, WP, C), np.uint8)
        pad[PAD:PAD + H, PAD:PAD + W] = _f32_to_e4m3_u8(
            np.transpose(x[b], (1, 2, 0)))
        flat = pad.reshape(cfg.NROWS, C)
        pair = np.zeros((cfg.NROWS, 2 * C), np.uint8)
        pair[:, :C] = flat
        pair[:cfg.NROWS - WP, C:] = flat[WP:]
        xcl_b[b] = pair
    for core in range(cfg.n_cores):
        b = core // cfg.halves
        half = core % cfg.halves
        h0 = half * cfg.RS
        npx = cfg.NPXP
        hs = np.full(npx, h0, np.int64)
        ws = np.zeros(npx, np.int64)
        ii = np.arange(cfg.NPX)
        hs[:cfg.NPX] = h0 + ii // W
        ws[:cfg.NPX] = ii % W
        offb = offset[b][:, hs, ws].astype(np.float32)  # [18, npx]
        oy = offb[0::2]                                  # [9, npx]
        ox = offb[1::2]
        iy = np.floor(oy)
        ix = np.floor(ox)
        wy = (oy - iy).astype(np.float32)
        wx = (ox - ix).astype(np.float32)
        ry = hs[None] + PAD + KH[:, None] + iy           # [9, npx]
        cx = np.clip(ws[None] + PAD + KW[:, None] + ix, 0, WP - 2)
        r0 = np.clip(ry, 0, WP - 2)
        idx0 = (r0 * WP + cx).astype(np.int16)           # [9, npx]
        i = np.arange(npx)
        idxd = np.zeros((128, 9, cfg.NM), np.int16)
        for r in range(8):
            idxd[i % 16 + 16 * r, :, i // 16] = idx0.T
        # corner order matches gathered row blocks [v00 | v10 | v01 | v11]
        mapd = np.zeros((128, 9, cfg.NBLK, 4), np.float32)
        mapd[i % 128, :, i // 128, 0] = ((1 - wy) * (1 - wx)).T
        mapd[i % 128, :, i // 128, 1] = (wy * (1 - wx)).T
        mapd[i % 128, :, i // 128, 2] = ((1 - wy) * wx).T
        mapd[i % 128, :, i // 128, 3] = (wy * wx).T
        in_maps.append({
            "xcl": xcl_b[b], "idxd": idxd, "mapd": mapd,
        })
    return in_maps


_NC_CACHE = {}


def get_nc(cfg: Cfg, debug_dump=False):
    key = (cfg.H, cfg.W, cfg.C, cfg.n_cores, debug_dump,
           cfg.POOL16, cfg.ACT16, cfg.GH)
    if key not in _NC_CACHE:
        _NC_CACHE[key] = build_nc(cfg, debug_dump=debug_dump)
    return _NC_CACHE[key]


def kernel(x, offset, w0, b0, trace=False, debug_dump=False):
    cfg = CFG
    x = np.asarray(x, np.float32)
    offset = np.asarray(offset, np.float32)
    w0 = np.asarray(w0, np.float32)
    b0 = np.asarray(b0, np.float32)
    nc = get_nc(cfg, debug_dump=debug_dump)
    in_maps = host_prep(cfg, x, offset)
    w0rep = np.ascontiguousarray(
        np.broadcast_to(w0.reshape(1, cfg.C), (128, cfg.C)), np.float32)
    b0rep = np.full((128, 1), float(b0[0]), np.float32)
    for m in in_maps:
        m["w0r"] = w0rep
        m["b0r"] = b0rep
    if trace:
        try:
            import antenv.axon_hooks  # noqa: F401
        except ImportError:
            trace = False
    res = run_bass_kernel_spmd(nc, in_maps, core_ids=list(range(cfg.n_cores)),
                               trace=trace)
    B, H, W = cfg.B, cfg.H, cfg.W
    out = np.zeros((B, 1, H, W), np.float32)
    for core in range(cfg.n_cores):
        b = core // cfg.halves
        half = core % cfg.halves
        h0 = half * cfg.RS
        o = res.results[core]["out"]              # [128, NBLK]
        o = o.T.reshape(-1)[:cfg.NPX].reshape(cfg.RS, W)
        out[b, 0, h0:h0 + cfg.RS] = o
    if trace or debug_dump:
        kernel.last_results = res
    return out
